# revision 6
# baseline (speedup 1.0000x reference)
"""Trainium2 Bass kernel for BlurredNoise: 128-filter 1D conv (K=5000) over
16 noise sequences, scaled per-filter.

Math: out[s, b, t] = sum_k noise[s, t+k] * F[b, k] * scale[b]
  s in [0,16) (= batch 2 x 8 noise channels), b in [0,128), t in [0,4096).

Mapping: data-parallel over the 16 sequences, 2 per NeuronCore. On each
core the conv is 40 accumulating 128x128x512 matmuls per output tile:
  k = 128*j + i,  lhsT_j[i, b] = F[b, 128j+i]*scale[b]  (prepped on host),
  rhs_j[i, t]    = X[t + 128j + i]   (slice of a Toeplitz band, host-built).
fp32r matmuls run at full PE rate (1 cycle/row at N=512) with ~fp32 accuracy.
"""

import numpy as np

import concourse.bacc as bacc
import concourse.mybir as mybir
from concourse.tile import TileContext
from concourse.bass_utils import run_bass_kernel_spmd

N_CORES = 8
BATCH = 2
NOISE_CH = 8
N_SEQ = BATCH * NOISE_CH          # 16
SEQ_PER_CORE = N_SEQ // N_CORES   # 2
T_IN = 9095
K_TAPS = 5000
T_OUT = 4096                      # T_IN - K_TAPS + 1
NJ = 40                           # ceil(5000/128)
K_PAD = NJ * 128                  # 5120
N_FILT = 128
NT = T_OUT // 512                 # 8 output tiles of 512
TAU = (NT - 1) * 512 + 512 + (NJ - 1) * 128   # 9088 Toeplitz band width
X_PAD = TAU + 128                 # 9216 >= 127 + 9087 + 1

_compiled_nc = None


def _build():
    nc = bacc.Bacc(name="blurred_noise")
    f32 = mybir.dt.float32
    f32r = mybir.dt.float32r

    xt = nc.dram_tensor("xt", [SEQ_PER_CORE, 128, TAU], f32r, kind="ExternalInput")
    w = nc.dram_tensor("w", [128, K_PAD], f32r, kind="ExternalInput")
    out = nc.dram_tensor("out", [SEQ_PER_CORE, 128, T_OUT], f32, kind="ExternalOutput")

    with TileContext(nc) as tc:
        with (
            tc.tile_pool(name="wpool", bufs=1) as wp,
            tc.tile_pool(name="xpool", bufs=2) as xp,
            tc.tile_pool(name="opool", bufs=4) as op,
            tc.tile_pool(name="psum", bufs=8, space="PSUM") as pp,
        ):
            wt = wp.tile([128, K_PAD], f32r)
            nc.sync.dma_start(out=wt[:], in_=w[:])
            for s in range(SEQ_PER_CORE):
                xs = xp.tile([128, TAU], f32r)
                nc.sync.dma_start(out=xs[:], in_=xt[s])
                for g in range(NT // 4):
                    ptiles = [
                        pp.tile([128, 512], f32, name=f"acc_{s}_{g}_{i}", tag="acc")
                        for i in range(4)
                    ]
                    for j in range(NJ):
                        lhsT = wt[:, j * 128:(j + 1) * 128]
                        for tt in range(4):
                            col0 = (g * 4 + tt) * 512 + j * 128
                            nc.tensor.matmul(
                                ptiles[tt][:],
                                lhsT,
                                xs[:, col0:col0 + 512],
                                start=(j == 0),
                                stop=(j == NJ - 1),
                            )
                    for tt in range(4):
                        t0 = (g * 4 + tt) * 512
                        ot = op.tile([128, 512], f32)
                        nc.vector.tensor_copy(ot[:], ptiles[tt][:])
                        nc.sync.dma_start(out=out[s][:, t0:t0 + 512], in_=ot[:])
    nc.compile()
    return nc


def _get_nc():
    global _compiled_nc
    if _compiled_nc is None:
        _compiled_nc = _build()
    return _compiled_nc


def _prep_inputs(noise, blur_filters, output_scale):
    noise = np.ascontiguousarray(np.asarray(noise, dtype=np.float32))
    F = np.asarray(blur_filters, dtype=np.float32)
    scale = np.asarray(output_scale, dtype=np.float32).reshape(N_FILT)

    # Fold the per-filter output scale into the filters, zero-pad taps to 5120,
    # and lay out as W[i, 128j + b] = F[b, 128j + i] (contraction dim on
    # partitions, filter dim on the matmul free axis).
    gain = 1.0 + 1.0 * (scale - 1.0)
    Fp = np.zeros((N_FILT, K_PAD), dtype=np.float32)
    Fp[:, :K_TAPS] = F * gain[:, None]
    W = np.ascontiguousarray(Fp.reshape(N_FILT, NJ, 128).transpose(2, 1, 0).reshape(128, NJ * 128))

    # Toeplitz band per sequence: xt[s, i, tau] = X[s, i + tau].
    Xflat = np.zeros((N_SEQ, X_PAD), dtype=np.float32)
    Xflat[:, :T_IN] = noise.reshape(N_SEQ, T_IN)
    sv = np.lib.stride_tricks.sliding_window_view(Xflat, TAU, axis=1)  # (16, 129, TAU)
    in_maps = []
    for c in range(N_CORES):
        xt = np.ascontiguousarray(
            sv[c * SEQ_PER_CORE:(c + 1) * SEQ_PER_CORE, :128, :]
        )  # (2, 128, TAU)
        in_maps.append({"xt": xt, "w": W})
    return in_maps


def _run(noise, blur_filters, output_scale, trace=False, tmpdir=None):
    in_maps = _prep_inputs(noise, blur_filters, output_scale)
    nc = _get_nc()
    res = run_bass_kernel_spmd(
        nc, in_maps, list(range(N_CORES)), trace=trace, tmpdir=tmpdir
    )
    outs = np.stack([res.results[c]["out"] for c in range(N_CORES)])  # (8, 2, 128, 4096)
    full = outs.reshape(BATCH, NOISE_CH, N_FILT, T_OUT).reshape(BATCH, NOISE_CH * N_FILT, T_OUT)
    return np.ascontiguousarray(full), res


def kernel(noise, blur_filters, output_scale):
    full, _ = _run(noise, blur_filters, output_scale)
    return full


# revision 7
# speedup vs baseline: 1.0791x; 1.0791x over previous
"""Trainium2 Bass kernel for BlurredNoise: 128-filter 1D conv (K=5000) over
16 noise sequences, scaled per-filter.

Math: out[s, b, t] = sum_k noise[s, t+k] * F[b, k] * scale[b]
  s in [0,16) (= batch 2 x 8 noise channels), b in [0,128), t in [0,4096).

Mapping: data-parallel over the 16 sequences, 2 per NeuronCore. On each
core the conv is 40 accumulating 128x128x512 matmuls per output tile:
  k = 128*j + i,  lhsT_j[i, b] = F[b, 128j+i]*scale[b]  (prepped on host),
  rhs_j[i, t]    = X[t + 128j + i]   (slice of a Toeplitz band, host-built).
fp32r matmuls run at full PE rate (1 cycle/row at N=512) with ~fp32 accuracy.
"""

import numpy as np

import concourse.bacc as bacc
import concourse.mybir as mybir
from concourse.tile import TileContext
from concourse.bass_utils import run_bass_kernel_spmd

N_CORES = 8
BATCH = 2
NOISE_CH = 8
N_SEQ = BATCH * NOISE_CH          # 16
SEQ_PER_CORE = N_SEQ // N_CORES   # 2
T_IN = 9095
K_TAPS = 5000
T_OUT = 4096                      # T_IN - K_TAPS + 1
NJ = 40                           # ceil(5000/128)
K_PAD = NJ * 128                  # 5120
N_FILT = 128
NT = T_OUT // 512                 # 8 output tiles of 512
TAU = (NT - 1) * 512 + 512 + (NJ - 1) * 128   # 9088 Toeplitz band width
X_PAD = TAU + 128                 # 9216 >= 127 + 9087 + 1

_compiled_nc = None


def _build():
    nc = bacc.Bacc(name="blurred_noise")
    f32 = mybir.dt.float32
    f32r = mybir.dt.float32r

    xt = nc.dram_tensor("xt", [SEQ_PER_CORE, 128, TAU], f32r, kind="ExternalInput")
    w = nc.dram_tensor("w", [128, K_PAD], f32r, kind="ExternalInput")
    out = nc.dram_tensor("out", [SEQ_PER_CORE, 128, T_OUT], f32, kind="ExternalOutput")

    with TileContext(nc) as tc:
        with (
            tc.tile_pool(name="wpool", bufs=1) as wp,
            tc.tile_pool(name="xpool", bufs=2) as xp,
            tc.tile_pool(name="opool", bufs=4) as op,
            tc.tile_pool(name="psum", bufs=8, space="PSUM") as pp,
        ):
            wt = wp.tile([128, K_PAD], f32r)
            # Chunked loads so the first matmuls only gate on the first slabs
            # (Tile tracks RAW deps at byte-range granularity).
            WCH = 1024
            nc.sync.dma_start(out=wt[:, 0:WCH], in_=w[:, 0:WCH])
            xtiles = []
            XSL = TAU // 8  # 1136
            for s in range(SEQ_PER_CORE):
                xs = xp.tile([128, TAU], f32r, name=f"xs{s}")
                xtiles.append(xs)
            for sl in range(2):
                c0 = sl * XSL
                nc.sync.dma_start(out=xtiles[0][:, c0:c0 + XSL], in_=xt[0][:, c0:c0 + XSL])
            # Warm the PE HAM clock-gate while the first x slabs stream in:
            # dead matmuls on the (already loaded) first weight chunk.
            warm = pp.tile([128, 512], f32, name="warm", tag="acc")
            for i in range(24):
                nc.tensor.matmul(
                    warm[:, 0:256], wt[:, 0:128], wt[:, 256:512],
                    start=True, stop=True,
                )
            for wc in range(1, K_PAD // WCH):
                nc.sync.dma_start(
                    out=wt[:, wc * WCH:(wc + 1) * WCH], in_=w[:, wc * WCH:(wc + 1) * WCH]
                )
            for s in range(SEQ_PER_CORE):
                for sl in range(2 if s == 0 else 0, 8):
                    c0 = sl * XSL
                    nc.sync.dma_start(
                        out=xtiles[s][:, c0:c0 + XSL], in_=xt[s][:, c0:c0 + XSL]
                    )
            for s in range(SEQ_PER_CORE):
                xs = xtiles[s]
                for g in range(NT // 4):
                    ptiles = [
                        pp.tile([128, 512], f32, name=f"acc_{s}_{g}_{i}", tag="acc")
                        for i in range(4)
                    ]
                    for j in range(NJ):
                        lhsT = wt[:, j * 128:(j + 1) * 128]
                        for tt in range(4):
                            col0 = (g * 4 + tt) * 512 + j * 128
                            nc.tensor.matmul(
                                ptiles[tt][:],
                                lhsT,
                                xs[:, col0:col0 + 512],
                                start=(j == 0),
                                stop=(j == NJ - 1),
                            )
                    for tt in range(4):
                        t0 = (g * 4 + tt) * 512
                        ot = op.tile([128, 512], f32)
                        nc.vector.tensor_copy(ot[:], ptiles[tt][:])
                        nc.sync.dma_start(out=out[s][:, t0:t0 + 512], in_=ot[:])
    nc.compile()
    return nc


def _get_nc():
    global _compiled_nc
    if _compiled_nc is None:
        _compiled_nc = _build()
    return _compiled_nc


def _prep_inputs(noise, blur_filters, output_scale):
    noise = np.ascontiguousarray(np.asarray(noise, dtype=np.float32))
    F = np.asarray(blur_filters, dtype=np.float32)
    scale = np.asarray(output_scale, dtype=np.float32).reshape(N_FILT)

    # Fold the per-filter output scale into the filters, zero-pad taps to 5120,
    # and lay out as W[i, 128j + b] = F[b, 128j + i] (contraction dim on
    # partitions, filter dim on the matmul free axis).
    gain = 1.0 + 1.0 * (scale - 1.0)
    Fp = np.zeros((N_FILT, K_PAD), dtype=np.float32)
    Fp[:, :K_TAPS] = F * gain[:, None]
    W = np.ascontiguousarray(Fp.reshape(N_FILT, NJ, 128).transpose(2, 1, 0).reshape(128, NJ * 128))

    # Toeplitz band per sequence: xt[s, i, tau] = X[s, i + tau].
    Xflat = np.zeros((N_SEQ, X_PAD), dtype=np.float32)
    Xflat[:, :T_IN] = noise.reshape(N_SEQ, T_IN)
    sv = np.lib.stride_tricks.sliding_window_view(Xflat, TAU, axis=1)  # (16, 129, TAU)
    in_maps = []
    for c in range(N_CORES):
        xt = np.ascontiguousarray(
            sv[c * SEQ_PER_CORE:(c + 1) * SEQ_PER_CORE, :128, :]
        )  # (2, 128, TAU)
        in_maps.append({"xt": xt, "w": W})
    return in_maps


def _run(noise, blur_filters, output_scale, trace=False, tmpdir=None):
    in_maps = _prep_inputs(noise, blur_filters, output_scale)
    nc = _get_nc()
    res = run_bass_kernel_spmd(
        nc, in_maps, list(range(N_CORES)), trace=trace, tmpdir=tmpdir
    )
    outs = np.stack([res.results[c]["out"] for c in range(N_CORES)])  # (8, 2, 128, 4096)
    full = outs.reshape(BATCH, NOISE_CH, N_FILT, T_OUT).reshape(BATCH, NOISE_CH * N_FILT, T_OUT)
    return np.ascontiguousarray(full), res


def kernel(noise, blur_filters, output_scale):
    full, _ = _run(noise, blur_filters, output_scale)
    return full
